# revision 66
# baseline (speedup 1.0000x reference)
"""Distributed attention-energy softmax kernel for 8 trn2 NeuronCores.

Math: reference computes
    energies = (enc @ W.T + b) @ h        # [S]
    attn     = softmax(energies)          # [1,1,S]
Algebraic rewrite: (enc @ W.T) @ h = enc @ (h^T W), and (b @ h) is a
constant added to every energy, which softmax is invariant to. So:
    v        = h^T W                      # [H]
    energies = enc @ v                    # [S]  (up to a constant shift)
    attn     = softmax(energies)

Sharding (8 cores):
  - enc [S=8192, H=2048] sharded along S: 1024 rows/core.
  - W sharded by COLUMNS: core c holds W[:, c*256:(c+1)*256] and computes
    its v slice on the PE as 32 narrow [128,1]-output matmuls (output
    free size 1 makes the fp32 matmuls ~free vs ~17us for the row
    orientation), then an AllGather of the slices yields full v.
  - v broadcast to 128 partitions with PE ones-matmuls (K=1) into PSUM
    for the DVE, plus an ACT copy to SBUF for POOL (GPSIMD can't read
    PSUM). A chain of dummy PE matmuls gated on the first enc tile keeps
    the PE p-state ramped so the broadcast runs at full clock.
  - energies: POOL multiplies tiles 0,1 and half of tile 7; DVE
    multiplies the rest; ACT accumulate-reduces seven products and DVE
    the two latest. Disjoint per-engine v copies, product tiles, and
    e-accumulators keep the tile scheduler from serializing one engine
    behind another through coalesced waits.
  - Global softmax: per-partition max, POOL partition_all_reduce for the
    cross-partition max and (rescaled) sum, one AllGather of the 8
    (m_i, s_i) pairs, local rescale.
  - DMA choreography: the cost model serializes all transfers on one
    shared device in arrival (FIFO) order, so per-engine issue chains
    (scheduler-order-only add_dep_helper edges) pin the order: SP issues
    W, enc t0, the v-slice write (whose SemWait gates later SP pieces),
    then more enc; POOL issues two pieces behind the v AllGather to fill
    the gap before the v read-back; ACT issues the v read-back then the
    remaining enc pieces, last tiles split in halves to cut the tail.

Layouts:
  - h input per core: [128, 16] with h_in[p, t] = h[t*128 + p] (replicated)
  - w input per core: [2048, 256] column slice of W
  - out per core: [128, 8] with out[p, t] = attn[core*1024 + t*128 + p]
"""

import numpy as np

H = 2048
S = 8192
N_CORES = 8
S_SHARD = S // N_CORES          # 1024
V_SHARD = H // N_CORES          # 256 v elements per core
N_TILES = S_SHARD // 128        # 8 row-tiles per core
N_CHUNKS = 8                    # enc DMA chunks (1 row-tile each)
KT = H // 128                   # 16 k-tiles for the v matvec

# enc DMA pieces: (row_tile, col0, col1, slot). Slots are per-engine
# issue chains whose SEQ order pins the shared DMA device's FIFO slots:
#   sp_pre  : SP, before the v-slice write
#   sp_post : SP, chained after the v-slice write
#   act_post: ACT, chained after the v read-back
ENC_PIECES = [
    (0, 0, 2048, "sp_pre"),
    (1, 0, 2048, "sp_post"),
    (2, 0, 1024, "pool_post"),
    (2, 1024, 1536, "pool_post"),
    (2, 1536, 2048, "act_post"),
    (3, 0, 2048, "act_post"),
    (4, 0, 2048, "act_post"),
    (5, 0, 2048, "act_post"),
    (7, 0, 1024, "act_post"),     # POOL's half of t7, delivered mid-stream
    (6, 0, 1024, "act_post"),
    (6, 1024, 2048, "act_post"),
    (7, 1024, 2048, "act_post"),
]
N_POOL_TILES = 2
POOL_T7_COLS = (0, 1024)
DVE_PIECES = {
    6: [(0, 1024), (1024, 2048)],
    7: [(1024, 2048)],
}
N_WARM = 7


def emit(tc, out_ap, enc_ap, w_ap, h_ap, local=False, gate=None):
    """Emit the per-core kernel IR into TileContext tc.

    out_ap: [128, 8] f32; enc_ap: [1024, 2048]; w_ap: [2048, 256];
    h_ap: [128, 16]. local=True replaces collectives with plain DMA
    copies (single-core timeline simulation only). gate: optional [128,1]
    tile AP from a previous emit; serializes this iteration behind it
    (benchmarking N-in-one-NEFF loops). Returns a [128,1] gate tile.
    """
    import concourse.bass_isa as bass_isa
    import concourse.mybir as mybir
    from concourse.tile_rust import add_dep_helper

    def chain(inst, prev):
        # sync=False: scheduler-order-only edge; the engine SEQ then
        # enforces issue (and so DMA-device FIFO) order at runtime.
        if prev is not None:
            add_dep_helper(inst.ins, prev.ins, sync=False, reason="dma order")
        return inst

    nc = tc.nc
    f32 = mybir.dt.float32
    rg = [list(range(N_CORES))]
    Exp = mybir.ActivationFunctionType.Exp
    X = mybir.AxisListType.X
    mult = mybir.AluOpType.mult
    add = mybir.AluOpType.add

    with (
        tc.tile_pool(name="psum", bufs=1, space="PSUM") as psum,
        tc.tile_pool(name="const", bufs=1) as const,
        tc.tile_pool(name="enc_p", bufs=N_TILES) as enc_p,
        tc.tile_pool(name="dram", bufs=1, space="DRAM") as dram,
    ):
        v_ps_a = psum.tile([128, H], f32)    # broadcast v (PE-written)
        vps = psum.tile([128, 2], f32)       # matvec v-slice

        ones_f32 = const.tile([1, 128], f32)
        nc.vector.memset(ones_f32[:], 1.0)

        # ---- input DMAs: ACT carries h, SP carries W then enc t0 ----
        h_sb = const.tile([128, KT], f32)
        nc.scalar.dma_start(h_sb[:], h_ap)
        w_sb = const.tile([128, KT, V_SHARD], f32)
        w_re = w_ap.rearrange("(t p) k -> p t k", p=128)
        nc.sync.dma_start(w_sb[:, 0:8, :], w_re[:, 0:8, :])
        nc.sync.dma_start(w_sb[:, 8:16, :], w_re[:, 8:16, :])

        if gate is not None:
            tok01 = const.tile([128, 1], f32)
            nc.vector.tensor_scalar(
                out=tok01[:], in0=gate, scalar1=0.0, scalar2=1.0,
                op0=mybir.AluOpType.mult, op1=mybir.AluOpType.add,
            )
            h_eff = const.tile([128, KT], f32)
            nc.vector.tensor_scalar_mul(h_eff[:], h_sb[:], tok01[:])
            h_sb = h_eff

        enc_tiles = [
            enc_p.tile([128, H], f32, tag="enc_c", name=f"enc_{u}")
            for u in range(N_TILES)
        ]

        def emit_enc_piece(u, c0, c1, eng, prev=None):
            enc_src = enc_ap[u * 128 : (u + 1) * 128, c0:c1]
            return chain(eng.dma_start(enc_tiles[u][:, c0:c1], enc_src), prev)

        for u, c0, c1, slot in ENC_PIECES:
            if slot == "sp_pre":
                emit_enc_piece(u, c0, c1, nc.sync)

        # ---- v slice: v[c*256 + ci*128 + p] as [128, 2] columns on the
        # PE; output free size 1 makes the fp32 matmuls ~free ----
        for ci in range(2):
            for t in range(KT):
                nc.tensor.matmul(
                    vps[:, ci : ci + 1],
                    lhsT=w_sb[:, t, ci * 128 : (ci + 1) * 128],
                    rhs=h_sb[:, t : t + 1],
                    start=(t == 0),
                    stop=(t == KT - 1),
                )
        v_sb = const.tile([128, 2], f32)
        nc.vector.tensor_copy(v_sb[:], vps[:])

        # ---- v AllGather: [2,128] slice -> [1,2048] full v. The SP
        # write's SemWait gates the SP-chained enc pieces' device slots.
        v_in_d = dram.tile([2, 128], f32)
        v_out_d = dram.tile([1, H], f32)
        prev = nc.sync.dma_start(v_in_d[:].rearrange("j p -> p j"), v_sb[:])
        for u, c0, c1, slot in ENC_PIECES:
            if slot == "sp_post":
                prev = emit_enc_piece(u, c0, c1, nc.sync, prev)
        if local:
            hop_b = nc.gpsimd.dma_start(
                v_out_d[0:1, 0:V_SHARD],
                v_in_d[:].rearrange("a b -> (a b)")[None, :],
            )
        else:
            hop_b = nc.gpsimd.collective_compute(
                "AllGather",
                mybir.AluOpType.bypass,
                replica_groups=rg,
                ins=[v_in_d.opt()],
                outs=[v_out_d.opt()],
            )
        # POOL: two enc pieces chained behind the AllGather fill the DMA
        # device between the AllGather and the v read-back.
        prev = hop_b
        for u, c0, c1, slot in ENC_PIECES:
            if slot == "pool_post":
                prev = emit_enc_piece(u, c0, c1, nc.gpsimd, prev)
        # ACT: read the gathered v as one [1, 2048] row, then stream the
        # remaining enc pieces behind it.
        v_row = const.tile([1, H], f32)
        prev = nc.scalar.dma_start(v_row[:], v_out_d[:])
        for u, c0, c1, slot in ENC_PIECES:
            if slot == "act_post":
                prev = emit_enc_piece(u, c0, c1, nc.scalar, prev)

        # ---- PE warm-up: cost is evaluated at dispatch and the p-state
        # ramps from the start of a continuous busy-run; a chain of
        # dummy matmuls gated on the first enc tile keeps the PE warm so
        # the v-broadcast matmuls run at full clock. ----
        for q in range(N_WARM):
            nc.tensor.matmul(
                v_ps_a[:, 0:512],
                lhsT=ones_f32[:],
                rhs=enc_tiles[0][0:1, 0:512],
                start=True,
                stop=True,
            )

        # ---- broadcast v across partitions: PE ones-matmuls into PSUM
        # for the DVE, then an ACT copy into SBUF for POOL (GPSIMD can't
        # read PSUM; disjoint copies keep DVE decoupled from POOL) ----
        for q in range(4):
            sl = slice(q * 512, (q + 1) * 512)
            nc.tensor.matmul(
                v_ps_a[:, sl],
                lhsT=ones_f32[:],
                rhs=v_row[0:1, sl],
                start=True,
                stop=True,
            )
        v_pool_sb = const.tile([128, H], f32)
        nc.scalar.activation(
            v_pool_sb[:],
            v_ps_a[:],
            mybir.ActivationFunctionType.Copy,
            bias=0.0,
            scale=1.0,
        )

        # ---- energies: POOL multiplies tiles 0,1 and t7's first half;
        # DVE multiplies the rest; ACT accumulate-reduces seven products,
        # DVE the two latest. Dedicated product tiles per tile avoid
        # WAR/WAW coupling between engines. ----
        n_dve = N_TILES - N_POOL_TILES
        e_pool = const.tile([128, N_POOL_TILES + 1], f32)  # +1: t7's POOL half
        e_dve = const.tile([128, n_dve], f32)
        prod_v = [const.tile([128, H], f32, name=f"prod_v{i}") for i in range(6)]
        prod_p = [const.tile([128, H], f32, name=f"prod_p{i}") for i in range(3)]
        act_scr = const.tile([128, H], f32)

        def act_reduce(prod, c0, c1, e_col):
            nc.scalar.activation(
                act_scr[:, 0 : c1 - c0],
                prod[:, c0:c1],
                mybir.ActivationFunctionType.Copy,
                bias=0.0,
                scale=1.0,
                accum_out=e_col,
            )

        def pool_mult(u, c0, c1, idx):
            prod = prod_p[idx]
            nc.gpsimd.tensor_tensor(
                out=prod[:, c0:c1],
                in0=enc_tiles[u][:, c0:c1],
                in1=v_pool_sb[:, c0:c1],
                op=mult,
            )
            return prod

        pp0 = pool_mult(0, 0, H, 0)
        pp1 = pool_mult(1, 0, H, 1)
        pp7 = pool_mult(7, *POOL_T7_COLS, 2)
        dve_prods = []
        for j, u in enumerate(range(N_POOL_TILES, N_TILES)):
            prod = prod_v[j]
            for c0, c1 in DVE_PIECES.get(u, [(0, H)]):
                nc.vector.tensor_tensor(
                    out=prod[:, c0:c1],
                    in0=enc_tiles[u][:, c0:c1],
                    in1=v_ps_a[:, c0:c1],
                    op=mult,
                )
            dve_prods.append((u, prod))
        act_reduce(dve_prods[0][1], 0, H, e_dve[:, 0:1])          # r2
        act_reduce(pp0, 0, H, e_pool[:, 0:1])                     # r0
        act_reduce(dve_prods[1][1], 0, H, e_dve[:, 1:2])          # r3
        act_reduce(dve_prods[2][1], 0, H, e_dve[:, 2:3])          # r4
        act_reduce(pp1, 0, H, e_pool[:, 1:2])                     # r1
        act_reduce(pp7, *POOL_T7_COLS, e_pool[:, 2:3])            # r7a
        act_reduce(dve_prods[4][1], 0, H, e_dve[:, 4:5])          # r6
        nc.vector.tensor_reduce(                                  # r5 (DVE)
            e_dve[:, 3:4], dve_prods[3][1][:], axis=X, op=add
        )
        e7b = const.tile([128, 1], f32)
        nc.vector.tensor_reduce(                                  # r7b (DVE)
            e7b[:], dve_prods[5][1][:, 1024:H], axis=X, op=add
        )
        nc.vector.tensor_tensor(                                  # e7 = a + b
            out=e_dve[:, 5:6], in0=e7b[:], in1=e_pool[:, 2:3], op=add
        )

        # ---- local softmax stats (split e groups: combine maxes) ----
        nm_d = const.tile([128, 1], f32)
        nc.vector.tensor_reduce(
            nm_d[:], e_dve[:], axis=X, op=mybir.AluOpType.max, negate=True
        )
        nm_p = const.tile([128, 1], f32)
        # only the full tiles; col 2 is t7's partial sum (covered by e_dve)
        nc.vector.tensor_reduce(
            nm_p[:], e_pool[:, 0:N_POOL_TILES], axis=X,
            op=mybir.AluOpType.max, negate=True,
        )
        nm_row = const.tile([128, 1], f32)  # -max_t e[p, t]
        nc.vector.tensor_tensor(
            out=nm_row[:], in0=nm_d[:], in1=nm_p[:], op=mybir.AluOpType.min
        )
        m_row = const.tile([128, 1], f32)
        nc.vector.tensor_scalar_mul(m_row[:], nm_row[:], -1.0)
        m_loc = const.tile([128, 1], f32)  # core max, all partitions
        nc.gpsimd.partition_all_reduce(
            m_loc[:], m_row[:], channels=128, reduce_op=bass_isa.ReduceOp.max
        )
        p_dve = const.tile([128, n_dve], f32)  # exp(e - m_p) per partition
        p_pool = const.tile([128, N_POOL_TILES], f32)
        s1 = const.tile([128, 1], f32)
        s2 = const.tile([128, 1], f32)
        nc.scalar.activation(
            p_dve[:], e_dve[:], Exp, bias=nm_row[:], scale=1.0, accum_out=s1[:]
        )
        nc.scalar.activation(
            p_pool[:], e_pool[:, 0:N_POOL_TILES], Exp,
            bias=nm_row[:], scale=1.0, accum_out=s2[:],
        )
        nm_loc = const.tile([128, 1], f32)
        nc.vector.tensor_scalar_mul(nm_loc[:], m_loc[:], -1.0)
        d_row = const.tile([128, 1], f32)  # exp(m_p - m_core)
        nc.scalar.activation(d_row[:], nm_row[:], Exp, bias=nm_loc[:], scale=-1.0)
        s_row = const.tile([128, 1], f32)
        nc.vector.tensor_tensor(out=s_row[:], in0=s1[:], in1=s2[:], op=add)
        t2 = const.tile([128, 1], f32)
        nc.vector.tensor_tensor(out=t2[:], in0=s_row[:], in1=d_row[:], op=mult)
        s_all = const.tile([128, 1], f32)  # core sum, all partitions
        nc.gpsimd.partition_all_reduce(
            s_all[:], t2[:], channels=128, reduce_op=bass_isa.ReduceOp.add
        )

        # ---- AllGather the (m_i, s_i) pairs ----
        st_sb = const.tile([1, 2], f32)
        nc.vector.tensor_copy(st_sb[0:1, 0:1], m_loc[0:1, :])
        nc.vector.tensor_copy(st_sb[0:1, 1:2], s_all[0:1, :])
        st_in_d = dram.tile([1, 2], f32)
        st_out_d = dram.tile([1, 2 * N_CORES], f32)
        nc.sync.dma_start(st_in_d[:], st_sb[:])
        if local:
            nc.gpsimd.dma_start(st_out_d[0:1, 0:2], st_in_d[:])
        else:
            nc.gpsimd.collective_compute(
                "AllGather",
                mybir.AluOpType.bypass,
                replica_groups=rg,
                ins=[st_in_d.opt()],
                outs=[st_out_d.opt()],
            )
        # stride-0 DMA broadcast of the 16 gathered stats to all partitions
        allst = const.tile([128, 2 * N_CORES], f32)
        nc.sync.dma_start(
            allst[:], st_out_d[0:1, :].broadcast_to([128, 2 * N_CORES])
        )

        # ---- combine: c_p = exp(m_p - gmax) / gsum ----
        m_vec = allst[:, 0 : 2 * N_CORES : 2]
        s_vec = allst[:, 1 : 2 * N_CORES : 2]
        red = const.tile([128, 1], f32)  # -gmax
        nc.vector.tensor_reduce(
            red[:], m_vec, axis=X, op=mybir.AluOpType.max, negate=True
        )
        t_vec = const.tile([128, N_CORES], f32)
        nc.scalar.activation(t_vec[:], m_vec, Exp, bias=red[:], scale=1.0)
        tmp_vec = const.tile([128, N_CORES], f32)
        nc.vector.tensor_tensor(out=tmp_vec[:], in0=t_vec[:], in1=s_vec, op=mult)
        gsum = const.tile([128, 1], f32)
        nc.vector.tensor_reduce(gsum[:], tmp_vec[:], axis=X, op=mybir.AluOpType.add)
        ginv = const.tile([128, 1], f32)
        nc.vector.reciprocal(ginv[:], gsum[:])
        cexp = const.tile([128, 1], f32)
        nc.scalar.activation(cexp[:], nm_row[:], Exp, bias=red[:], scale=-1.0)

        # ---- finalize: attn_shard = exp(e - m_p) * exp(m_p - gmax) / gsum,
        # fused as one tensor_scalar with two per-partition scalars ----
        o_sb = const.tile([128, N_TILES], f32)
        nc.vector.tensor_scalar(
            out=o_sb[:, 0:N_POOL_TILES], in0=p_pool[:],
            scalar1=cexp[:], scalar2=ginv[:], op0=mult, op1=mult,
        )
        nc.vector.tensor_scalar(
            out=o_sb[:, N_POOL_TILES:N_TILES], in0=p_dve[:],
            scalar1=cexp[:], scalar2=ginv[:], op0=mult, op1=mult,
        )
        nc.sync.dma_start(out_ap, o_sb[:])
        gate_out = const.tile([128, 1], f32)
        nc.vector.tensor_reduce(
            gate_out[:], o_sb[:], axis=X, op=mybir.AluOpType.max
        )
        return gate_out[:]


def _build_bass():
    import concourse.bacc as bacc
    import concourse.mybir as mybir
    import concourse.tile as tile

    f32 = mybir.dt.float32
    nc = bacc.Bacc(
        "TRN2", target_bir_lowering=False, debug=False, num_devices=N_CORES
    )
    enc_in = nc.dram_tensor("enc", [S_SHARD, H], f32, kind="ExternalInput")
    w_in = nc.dram_tensor("w", [H, V_SHARD], f32, kind="ExternalInput")
    h_in = nc.dram_tensor("h", [128, KT], f32, kind="ExternalInput")
    out = nc.dram_tensor("attn", [128, N_TILES], f32, kind="ExternalOutput")

    with tile.TileContext(nc) as tc:
        emit(tc, out.ap(), enc_in.ap(), w_in.ap(), h_in.ap())

    nc.compile()
    return nc


_NC_CACHE = None


def make_in_maps(hidden, encoder_outputs, W):
    h = np.asarray(hidden, dtype=np.float32).reshape(H)
    enc = np.asarray(encoder_outputs, dtype=np.float32).reshape(S, H)
    W = np.asarray(W, dtype=np.float32)
    h_tile = np.ascontiguousarray(h.reshape(KT, 128).T)
    in_maps = []
    for c in range(N_CORES):
        in_maps.append(
            {
                "enc": np.ascontiguousarray(enc[c * S_SHARD : (c + 1) * S_SHARD]),
                "w": np.ascontiguousarray(W[:, c * V_SHARD : (c + 1) * V_SHARD]),
                "h": h_tile,
            }
        )
    return in_maps


def kernel(hidden, encoder_outputs, W, b):
    from concourse import bass_utils

    global _NC_CACHE
    if _NC_CACHE is None:
        _NC_CACHE = _build_bass()
    nc = _NC_CACHE

    in_maps = make_in_maps(hidden, encoder_outputs, W)
    res = bass_utils.run_bass_kernel_spmd(
        nc, in_maps, core_ids=list(range(N_CORES))
    )
    shards = [r["attn"].T.reshape(S_SHARD) for r in res.results]
    return np.concatenate(shards).reshape(1, 1, S).astype(np.float32)


# revision 81
# speedup vs baseline: 1.0423x; 1.0423x over previous
"""Distributed attention-energy softmax kernel for 8 trn2 NeuronCores.

Math: reference computes
    energies = (enc @ W.T + b) @ h        # [S]
    attn     = softmax(energies)          # [1,1,S]
Algebraic rewrite: (enc @ W.T) @ h = enc @ (h^T W), and (b @ h) is a
constant added to every energy, which softmax is invariant to. So:
    v        = h^T W                      # [H]
    energies = enc @ v                    # [S]  (up to a constant shift)
    attn     = softmax(energies)

Sharding (8 cores):
  - enc [S=8192, H=2048] sharded along S: 1024 rows/core.
  - W sharded by COLUMNS: core c holds W[:, c*256:(c+1)*256] and computes
    its v slice on the PE as 32 narrow [128,1]-output matmuls (output
    free size 1 makes the fp32 matmuls ~free vs ~17us for the row
    orientation), then an AllGather of the slices yields full v.
  - v broadcast to 128 partitions with PE ones-matmuls (K=1) into PSUM
    for the DVE, plus an ACT copy to SBUF for POOL (GPSIMD can't read
    PSUM). A chain of dummy PE matmuls gated on the first enc tile keeps
    the PE p-state ramped so the broadcast runs at full clock.
  - energies: POOL multiplies tiles 0,1 and half of tile 7; DVE
    multiplies the rest; ACT accumulate-reduces seven products and DVE
    the two latest. Disjoint per-engine v copies, product tiles, and
    e-accumulators keep the tile scheduler from serializing one engine
    behind another through coalesced waits.
  - Global softmax: per-partition max, POOL partition_all_reduce for the
    cross-partition max and (rescaled) sum, one AllGather of the 8
    (m_i, s_i) pairs, local rescale.
  - DMA choreography: the cost model serializes all transfers on one
    shared device in arrival (FIFO) order, so per-engine issue chains
    (scheduler-order-only add_dep_helper edges) pin the order: SP issues
    W, enc t0, the v-slice write (whose SemWait gates later SP pieces),
    then more enc; POOL issues two pieces behind the v AllGather to fill
    the gap before the v read-back; ACT issues the v read-back then the
    remaining enc pieces, last tiles split in halves to cut the tail.

Layouts:
  - h input per core: [128, 16] with h_in[p, t] = h[t*128 + p] (replicated)
  - w input per core: [2048, 256] column slice of W
  - out per core: [128, 8] with out[p, t] = attn[core*1024 + t*128 + p]
"""

import numpy as np

H = 2048
S = 8192
N_CORES = 8
S_SHARD = S // N_CORES          # 1024
V_SHARD = H // N_CORES          # 256 v elements per core
N_TILES = S_SHARD // 128        # 8 row-tiles per core
N_CHUNKS = 8                    # enc DMA chunks (1 row-tile each)
KT = H // 128                   # 16 k-tiles for the v matvec

# enc DMA pieces: (row_tile, col0, col1, slot). Slots are per-engine
# issue chains whose SEQ order pins the shared DMA device's FIFO slots:
#   sp_pre  : SP, before the v-slice write
#   sp_post : SP, chained after the v-slice write
#   act_post: ACT, chained after the v read-back
ENC_PIECES = [
    (0, 0, 2048, "sp_pre"),
    (1, 0, 2048, "sp_post"),
    (2, 0, 1024, "pool_post"),
    (2, 1024, 1536, "pool_post"),
    (2, 1536, 2048, "act_post"),
    (3, 0, 2048, "act_post"),
    (4, 0, 2048, "act_post"),
    (5, 0, 2048, "act_post"),
    (7, 0, 1024, "act_post"),     # POOL's half of t7, delivered mid-stream
    (6, 0, 1024, "act_post"),
    (6, 1024, 2048, "act_post"),
    (7, 1024, 2048, "act_post"),
]
N_POOL_TILES = 2
POOL_T7_COLS = (0, 1024)
DVE_PIECES = {
    6: [(0, 1024), (1024, 2048)],
    7: [(1024, 2048)],
}
N_WARM = 4


def emit(tc, out_ap, enc_ap, w_ap, h_ap, local=False, gate=None):
    """Emit the per-core kernel IR into TileContext tc.

    out_ap: [128, 8] f32; enc_ap: [1024, 2048]; w_ap: [2048, 256];
    h_ap: [128, 16]. local=True replaces collectives with plain DMA
    copies (single-core timeline simulation only). gate: optional [128,1]
    tile AP from a previous emit; serializes this iteration behind it
    (benchmarking N-in-one-NEFF loops). Returns a [128,1] gate tile.
    """
    import concourse.bass_isa as bass_isa
    import concourse.mybir as mybir
    from concourse.tile_rust import add_dep_helper

    def chain(inst, prev):
        # sync=False: scheduler-order-only edge; the engine SEQ then
        # enforces issue (and so DMA-device FIFO) order at runtime.
        if prev is not None:
            add_dep_helper(inst.ins, prev.ins, sync=False, reason="dma order")
        return inst

    nc = tc.nc
    f32 = mybir.dt.float32
    rg = [list(range(N_CORES))]
    Exp = mybir.ActivationFunctionType.Exp
    X = mybir.AxisListType.X
    mult = mybir.AluOpType.mult
    add = mybir.AluOpType.add

    with (
        tc.tile_pool(name="psum", bufs=1, space="PSUM") as psum,
        tc.tile_pool(name="const", bufs=1) as const,
        tc.tile_pool(name="enc_p", bufs=N_TILES) as enc_p,
        tc.tile_pool(name="dram", bufs=1, space="DRAM") as dram,
    ):
        v_ps_a = psum.tile([128, H], f32)    # broadcast v (PE-written)
        vps = psum.tile([128, 2], f32)       # matvec v-slice

        ones_f32 = const.tile([1, 128], f32)
        nc.vector.memset(ones_f32[:], 1.0)

        # ---- input DMAs: ACT carries h, SP carries W then enc t0 ----
        h_sb = const.tile([128, KT], f32)
        nc.scalar.dma_start(h_sb[:], h_ap)
        w_sb = const.tile([128, KT, V_SHARD], f32)
        w_re = w_ap.rearrange("(t p) k -> p t k", p=128)
        nc.sync.dma_start(w_sb[:, 0:8, :], w_re[:, 0:8, :])
        nc.sync.dma_start(w_sb[:, 8:16, :], w_re[:, 8:16, :])

        if gate is not None:
            tok01 = const.tile([128, 1], f32)
            nc.vector.tensor_scalar(
                out=tok01[:], in0=gate, scalar1=0.0, scalar2=1.0,
                op0=mybir.AluOpType.mult, op1=mybir.AluOpType.add,
            )
            h_eff = const.tile([128, KT], f32)
            nc.vector.tensor_scalar_mul(h_eff[:], h_sb[:], tok01[:])
            h_sb = h_eff

        enc_tiles = [
            enc_p.tile([128, H], f32, tag="enc_c", name=f"enc_{u}")
            for u in range(N_TILES)
        ]

        def emit_enc_piece(u, c0, c1, eng, prev=None):
            enc_src = enc_ap[u * 128 : (u + 1) * 128, c0:c1]
            return chain(eng.dma_start(enc_tiles[u][:, c0:c1], enc_src), prev)

        for u, c0, c1, slot in ENC_PIECES:
            if slot == "sp_pre":
                emit_enc_piece(u, c0, c1, nc.sync)

        # ---- v slice: v[c*256 + ci*128 + p] as [128, 2] columns on the
        # PE; output free size 1 makes the fp32 matmuls ~free ----
        for ci in range(2):
            for t in range(KT):
                nc.tensor.matmul(
                    vps[:, ci : ci + 1],
                    lhsT=w_sb[:, t, ci * 128 : (ci + 1) * 128],
                    rhs=h_sb[:, t : t + 1],
                    start=(t == 0),
                    stop=(t == KT - 1),
                )
        v_sb = const.tile([128, 2], f32)
        nc.vector.tensor_copy(v_sb[:], vps[:])

        # ---- v AllGather: [2,128] slice -> [1,2048] full v. The SP
        # write's SemWait gates the SP-chained enc pieces' device slots.
        v_in_d = dram.tile([2, 128], f32)
        v_out_d = dram.tile([1, H], f32)
        prev = nc.sync.dma_start(v_in_d[:].rearrange("j p -> p j"), v_sb[:])
        for u, c0, c1, slot in ENC_PIECES:
            if slot == "sp_post":
                prev = emit_enc_piece(u, c0, c1, nc.sync, prev)
        if local:
            hop_b = nc.gpsimd.dma_start(
                v_out_d[0:1, 0:V_SHARD],
                v_in_d[:].rearrange("a b -> (a b)")[None, :],
            )
        else:
            hop_b = nc.gpsimd.collective_compute(
                "AllGather",
                mybir.AluOpType.bypass,
                replica_groups=rg,
                ins=[v_in_d.opt()],
                outs=[v_out_d.opt()],
            )
        # POOL: two enc pieces chained behind the AllGather fill the DMA
        # device between the AllGather and the v read-back.
        prev = hop_b
        for u, c0, c1, slot in ENC_PIECES:
            if slot == "pool_post":
                prev = emit_enc_piece(u, c0, c1, nc.gpsimd, prev)
        # ACT: read the gathered v as one [1, 2048] row, then stream the
        # remaining enc pieces behind it.
        v_row = const.tile([1, H], f32)
        prev = nc.scalar.dma_start(v_row[:], v_out_d[:])
        for u, c0, c1, slot in ENC_PIECES:
            if slot == "act_post":
                prev = emit_enc_piece(u, c0, c1, nc.scalar, prev)

        # ---- PE warm-up: cost is evaluated at dispatch and the p-state
        # ramps from the start of a continuous busy-run; a chain of
        # dummy matmuls gated on the first enc tile keeps the PE warm so
        # the v-broadcast matmuls run at full clock. ----
        for q in range(N_WARM):
            nc.tensor.matmul(
                v_ps_a[:, 0:512],
                lhsT=ones_f32[:],
                rhs=enc_tiles[0][0:1, 0:512],
                start=True,
                stop=True,
            )

        # ---- broadcast v across partitions: PE ones-matmuls into PSUM
        # for the DVE, then an ACT copy into SBUF for POOL (GPSIMD can't
        # read PSUM; disjoint copies keep DVE decoupled from POOL) ----
        for q in range(4):
            sl = slice(q * 512, (q + 1) * 512)
            nc.tensor.matmul(
                v_ps_a[:, sl],
                lhsT=ones_f32[:],
                rhs=v_row[0:1, sl],
                start=True,
                stop=True,
            )
        v_pool_sb = const.tile([128, H], f32)
        nc.gpsimd.partition_broadcast(v_pool_sb[:], v_row[:])

        # ---- energies: POOL multiplies tiles 0,1 and t7's first half;
        # DVE multiplies the rest; ACT accumulate-reduces seven products,
        # DVE the two latest. Dedicated product tiles per tile avoid
        # WAR/WAW coupling between engines. ----
        n_dve = N_TILES - N_POOL_TILES
        e_pool = const.tile([128, N_POOL_TILES + 1], f32)  # +1: t7's POOL half
        e_dve = const.tile([128, n_dve], f32)
        prod_v = [const.tile([128, H], f32, name=f"prod_v{i}") for i in range(6)]
        prod_p = [const.tile([128, H], f32, name=f"prod_p{i}") for i in range(3)]
        act_scr = const.tile([128, H], f32)

        def act_reduce(prod, c0, c1, e_col):
            nc.scalar.activation(
                act_scr[:, 0 : c1 - c0],
                prod[:, c0:c1],
                mybir.ActivationFunctionType.Copy,
                bias=0.0,
                scale=1.0,
                accum_out=e_col,
            )

        def pool_mult(u, c0, c1, idx):
            prod = prod_p[idx]
            nc.gpsimd.tensor_tensor(
                out=prod[:, c0:c1],
                in0=enc_tiles[u][:, c0:c1],
                in1=v_pool_sb[:, c0:c1],
                op=mult,
            )
            return prod

        pp0 = pool_mult(0, 0, H, 0)
        pp1 = pool_mult(1, 0, H, 1)
        pp7 = pool_mult(7, *POOL_T7_COLS, 2)
        dve_prods = []
        for j, u in enumerate(range(N_POOL_TILES, N_TILES)):
            prod = prod_v[j]
            for c0, c1 in DVE_PIECES.get(u, [(0, H)]):
                nc.vector.tensor_tensor(
                    out=prod[:, c0:c1],
                    in0=enc_tiles[u][:, c0:c1],
                    in1=v_ps_a[:, c0:c1],
                    op=mult,
                )
            dve_prods.append((u, prod))
        act_reduce(dve_prods[0][1], 0, H, e_dve[:, 0:1])          # r2
        act_reduce(pp0, 0, H, e_pool[:, 0:1])                     # r0
        act_reduce(dve_prods[1][1], 0, H, e_dve[:, 1:2])          # r3
        act_reduce(dve_prods[2][1], 0, H, e_dve[:, 2:3])          # r4
        act_reduce(pp1, 0, H, e_pool[:, 1:2])                     # r1
        act_reduce(pp7, *POOL_T7_COLS, e_pool[:, 2:3])            # r7a
        act_reduce(dve_prods[4][1], 0, H, e_dve[:, 4:5])          # r6
        nc.vector.tensor_reduce(                                  # r5 (DVE)
            e_dve[:, 3:4], dve_prods[3][1][:], axis=X, op=add
        )
        e7b = const.tile([128, 1], f32)
        nc.vector.tensor_reduce(                                  # r7b (DVE)
            e7b[:], dve_prods[5][1][:, 1024:H], axis=X, op=add
        )
        nc.vector.tensor_tensor(                                  # e7 = a + b
            out=e_dve[:, 5:6], in0=e7b[:], in1=e_pool[:, 2:3], op=add
        )

        # ---- local softmax stats (split e groups: combine maxes) ----
        nm_d = const.tile([128, 1], f32)
        nc.vector.tensor_reduce(
            nm_d[:], e_dve[:], axis=X, op=mybir.AluOpType.max, negate=True
        )
        nm_p = const.tile([128, 1], f32)
        # only the full tiles; col 2 is t7's partial sum (covered by e_dve)
        nc.vector.tensor_reduce(
            nm_p[:], e_pool[:, 0:N_POOL_TILES], axis=X,
            op=mybir.AluOpType.max, negate=True,
        )
        nm_row = const.tile([128, 1], f32)  # -max_t e[p, t]
        nc.vector.tensor_tensor(
            out=nm_row[:], in0=nm_d[:], in1=nm_p[:], op=mybir.AluOpType.min
        )
        m_row = const.tile([128, 1], f32)
        nc.vector.tensor_scalar_mul(m_row[:], nm_row[:], -1.0)
        m_loc = const.tile([128, 1], f32)  # core max, all partitions
        nc.gpsimd.partition_all_reduce(
            m_loc[:], m_row[:], channels=128, reduce_op=bass_isa.ReduceOp.max
        )
        p_dve = const.tile([128, n_dve], f32)  # exp(e - m_p) per partition
        p_pool = const.tile([128, N_POOL_TILES], f32)
        s1 = const.tile([128, 1], f32)
        s2 = const.tile([128, 1], f32)
        nc.scalar.activation(
            p_dve[:], e_dve[:], Exp, bias=nm_row[:], scale=1.0, accum_out=s1[:]
        )
        nc.scalar.activation(
            p_pool[:], e_pool[:, 0:N_POOL_TILES], Exp,
            bias=nm_row[:], scale=1.0, accum_out=s2[:],
        )
        nm_loc = const.tile([128, 1], f32)
        nc.vector.tensor_scalar_mul(nm_loc[:], m_loc[:], -1.0)
        d_row = const.tile([128, 1], f32)  # exp(m_p - m_core)
        nc.scalar.activation(d_row[:], nm_row[:], Exp, bias=nm_loc[:], scale=-1.0)
        s_row = const.tile([128, 1], f32)
        nc.vector.tensor_tensor(out=s_row[:], in0=s1[:], in1=s2[:], op=add)
        t2 = const.tile([128, 1], f32)
        nc.vector.tensor_tensor(out=t2[:], in0=s_row[:], in1=d_row[:], op=mult)
        s_all = const.tile([128, 1], f32)  # core sum, all partitions
        nc.gpsimd.partition_all_reduce(
            s_all[:], t2[:], channels=128, reduce_op=bass_isa.ReduceOp.add
        )

        # ---- AllGather the (m_i, s_i) pairs ----
        st_sb = const.tile([1, 2], f32)
        nc.vector.tensor_copy(st_sb[0:1, 0:1], m_loc[0:1, :])
        nc.vector.tensor_copy(st_sb[0:1, 1:2], s_all[0:1, :])
        st_in_d = dram.tile([1, 2], f32)
        st_out_d = dram.tile([1, 2 * N_CORES], f32)
        nc.sync.dma_start(st_in_d[:], st_sb[:])
        if local:
            nc.gpsimd.dma_start(st_out_d[0:1, 0:2], st_in_d[:])
        else:
            nc.gpsimd.collective_compute(
                "AllGather",
                mybir.AluOpType.bypass,
                replica_groups=rg,
                ins=[st_in_d.opt()],
                outs=[st_out_d.opt()],
            )
        # stride-0 DMA broadcast of the 16 gathered stats to all partitions
        allst = const.tile([128, 2 * N_CORES], f32)
        nc.sync.dma_start(
            allst[:], st_out_d[0:1, :].broadcast_to([128, 2 * N_CORES])
        )

        # ---- combine: c_p = exp(m_p - gmax) / gsum ----
        m_vec = allst[:, 0 : 2 * N_CORES : 2]
        s_vec = allst[:, 1 : 2 * N_CORES : 2]
        red = const.tile([128, 1], f32)  # -gmax
        nc.vector.tensor_reduce(
            red[:], m_vec, axis=X, op=mybir.AluOpType.max, negate=True
        )
        t_vec = const.tile([128, N_CORES], f32)
        nc.scalar.activation(t_vec[:], m_vec, Exp, bias=red[:], scale=1.0)
        tmp_vec = const.tile([128, N_CORES], f32)
        nc.vector.tensor_tensor(out=tmp_vec[:], in0=t_vec[:], in1=s_vec, op=mult)
        gsum = const.tile([128, 1], f32)
        nc.vector.tensor_reduce(gsum[:], tmp_vec[:], axis=X, op=mybir.AluOpType.add)
        ginv = const.tile([128, 1], f32)
        nc.vector.reciprocal(ginv[:], gsum[:])
        cexp = const.tile([128, 1], f32)
        nc.scalar.activation(cexp[:], nm_row[:], Exp, bias=red[:], scale=-1.0)

        # ---- finalize: attn_shard = exp(e - m_p) * exp(m_p - gmax) / gsum,
        # fused as one tensor_scalar with two per-partition scalars ----
        o_sb = const.tile([128, N_TILES], f32)
        nc.vector.tensor_scalar(
            out=o_sb[:, 0:N_POOL_TILES], in0=p_pool[:],
            scalar1=cexp[:], scalar2=ginv[:], op0=mult, op1=mult,
        )
        nc.vector.tensor_scalar(
            out=o_sb[:, N_POOL_TILES:N_TILES], in0=p_dve[:],
            scalar1=cexp[:], scalar2=ginv[:], op0=mult, op1=mult,
        )
        nc.sync.dma_start(out_ap, o_sb[:])
        gate_out = const.tile([128, 1], f32)
        nc.vector.tensor_reduce(
            gate_out[:], o_sb[:], axis=X, op=mybir.AluOpType.max
        )
        return gate_out[:]


def _build_bass():
    import concourse.bacc as bacc
    import concourse.mybir as mybir
    import concourse.tile as tile

    f32 = mybir.dt.float32
    nc = bacc.Bacc(
        "TRN2", target_bir_lowering=False, debug=False, num_devices=N_CORES
    )
    enc_in = nc.dram_tensor("enc", [S_SHARD, H], f32, kind="ExternalInput")
    w_in = nc.dram_tensor("w", [H, V_SHARD], f32, kind="ExternalInput")
    h_in = nc.dram_tensor("h", [128, KT], f32, kind="ExternalInput")
    out = nc.dram_tensor("attn", [128, N_TILES], f32, kind="ExternalOutput")

    with tile.TileContext(nc) as tc:
        emit(tc, out.ap(), enc_in.ap(), w_in.ap(), h_in.ap())

    nc.compile()
    return nc


_NC_CACHE = None


def make_in_maps(hidden, encoder_outputs, W):
    h = np.asarray(hidden, dtype=np.float32).reshape(H)
    enc = np.asarray(encoder_outputs, dtype=np.float32).reshape(S, H)
    W = np.asarray(W, dtype=np.float32)
    h_tile = np.ascontiguousarray(h.reshape(KT, 128).T)
    in_maps = []
    for c in range(N_CORES):
        in_maps.append(
            {
                "enc": np.ascontiguousarray(enc[c * S_SHARD : (c + 1) * S_SHARD]),
                "w": np.ascontiguousarray(W[:, c * V_SHARD : (c + 1) * V_SHARD]),
                "h": h_tile,
            }
        )
    return in_maps


def kernel(hidden, encoder_outputs, W, b):
    from concourse import bass_utils

    global _NC_CACHE
    if _NC_CACHE is None:
        _NC_CACHE = _build_bass()
    nc = _NC_CACHE

    in_maps = make_in_maps(hidden, encoder_outputs, W)
    res = bass_utils.run_bass_kernel_spmd(
        nc, in_maps, core_ids=list(range(N_CORES))
    )
    shards = [r["attn"].T.reshape(S_SHARD) for r in res.results]
    return np.concatenate(shards).reshape(1, 1, S).astype(np.float32)
